# revision 3
# baseline (speedup 1.0000x reference)
"""DropStripes (dim=2 SpecAugment) Trainium2 Bass kernel.

x: [64, 1, 4096, 256] f32; bgn, distance: [64, 2] i32.
Zero time stripes [bgn, bgn+distance) along axis 2 per sample.
Pure data parallel over batch across 8 NeuronCores (8 samples/core).

Per-core program (~118-122us typical vs ~202-218us for the previous
SBUF-bounce+multiply streaming kernel):
  - 8 x 4MiB DRAM->DRAM copies x[b] -> out[b], alternating the two HWDGE
    queues, issued up front with no waits. No SBUF bounce and no vector
    multiply, so each byte crosses the SDMA engines once instead of
    twice; measured ~21 B/ns on each of the 16 SDMA engines at 98-99%
    duty (~94% of the per-core HBM read+write wall).
  - stripe zeroing via gpsimd indirect scatters of a zeros row [256] f32
    to up to 126 dynamic row indices per sample (both stripes; padding
    indices > bounds_check are skipped, oob_is_err=False). Indices are
    computed on-device from bgn/distance with a short int32 vector
    chain.
  - fixup placement is hybrid: samples 0-5 scatter into `out` right
    after their own copy completes (hidden under the stream); samples
    6-7 instead pre-poison x early (~20us), before the tail of the ring
    ever reads x[6:8], so the last copies carry the zeros and there is
    no scatter tail after the stream.
  - the dependency tracker treats each dynamic-AP scatter as touching
    the whole tensor, which would serialize copies behind scatters;
    those false deps are pruned to the true ones (out-scatter b after
    copy b, copy 6/7 after x-scatter 6/7). Valid scatter indices stay
    inside sample b's rows, so cross-sample deps are value-benign.
"""
import numpy as np

B, C, T, F = 64, 1, 4096, 256
S = 2
N_CORES = 8
BL = B // N_CORES          # samples per core
BIG = 1 << 16              # OOB padding row index

_cached_nc = None


def _raw(i):
    return i.ins if hasattr(i, "ins") else i


def _build():
    from contextlib import ExitStack
    import concourse.tile as tile
    from concourse import bacc, bass, mybir

    nc = bacc.Bacc("TRN2", target_bir_lowering=False, debug=False)
    x_d = nc.dram_tensor("x", [BL, T, F], mybir.dt.float32, kind="ExternalInput")
    bgn_d = nc.dram_tensor("bgn", [BL, S], mybir.dt.int32, kind="ExternalInput")
    dist_d = nc.dram_tensor("distance", [BL, S], mybir.dt.int32, kind="ExternalInput")
    # constant table: col 0 = partition index p; col 1+b = b*T
    tab_d = nc.dram_tensor("tab", [128, 1 + BL], mybir.dt.int32, kind="ExternalInput")
    out_d = nc.dram_tensor("out", [BL, T, F], mybir.dt.float32, kind="ExternalOutput")

    with tile.TileContext(nc) as tc, ExitStack() as ctx:
        pool = ctx.enter_context(tc.tile_pool(name="small", bufs=1))

        # --- tiny input DMAs on the gpsimd (SWDGE) queue, keeping both
        # HWDGE queues free for the copy stream.
        tab = pool.tile([128, 1 + BL], mybir.dt.int32)
        nc.gpsimd.dma_start(tab[:, :], tab_d[:])
        bgn_bc = pool.tile([128, BL * S], mybir.dt.int32)
        nc.gpsimd.dma_start(
            bgn_bc[:, :], bgn_d[:].flatten().unsqueeze(0).broadcast_to([128, BL * S]))
        dist_bc = pool.tile([128, BL * S], mybir.dt.int32)
        nc.gpsimd.dma_start(
            dist_bc[:, :], dist_d[:].flatten().unsqueeze(0).broadcast_to([128, BL * S]))

        # --- row-index computation, all [128, BL] int32
        # idx[p, b] = b*T + (bgn0+p          if p < d0
        #                    bgn1 + (p - d0) if p - d0 < d1
        #                    BIG             otherwise)
        P_ = tab[:, 0:1].broadcast_to([128, BL])
        OFF = tab[:, 1:1 + BL]
        b0 = bgn_bc[:, 0::S]
        b1 = bgn_bc[:, 1::S]
        d0 = dist_bc[:, 0::S]
        d1 = dist_bc[:, 1::S]

        def tmp(name):
            return pool.tile([128, BL], mybir.dt.int32, name=name)

        t0 = tmp("t0")
        c0 = tmp("c0")
        pm = tmp("pm")
        c1 = tmp("c1")
        t1 = tmp("t1")
        s1 = tmp("s1")
        w = tmp("w")
        idx = tmp("idx")
        nc.vector.tensor_add(t0[:, :], b0, P_)
        nc.vector.tensor_tensor(c0[:, :], P_, d0, op=mybir.AluOpType.is_lt)
        nc.vector.tensor_sub(pm[:, :], P_, d0)
        nc.vector.tensor_tensor(c1[:, :], pm[:, :], d1, op=mybir.AluOpType.is_lt)
        nc.vector.tensor_add(t1[:, :], b1, pm[:, :])
        # s1 = c1 ? t1 : BIG  ==  c1*(t1-BIG) + BIG
        nc.vector.tensor_scalar(t1[:, :], t1[:, :], -BIG, None, op0=mybir.AluOpType.add)
        nc.vector.tensor_tensor(s1[:, :], c1[:, :], t1[:, :], op=mybir.AluOpType.mult)
        nc.vector.tensor_scalar(s1[:, :], s1[:, :], BIG, None, op0=mybir.AluOpType.add)
        # idx_local = c0 ? t0 : s1  ==  c0*(t0-s1) + s1
        nc.vector.tensor_sub(w[:, :], t0[:, :], s1[:, :])
        nc.vector.tensor_tensor(w[:, :], c0[:, :], w[:, :], op=mybir.AluOpType.mult)
        nc.vector.tensor_add(idx[:, :], w[:, :], s1[:, :])
        nc.vector.tensor_add(idx[:, :], idx[:, :], OFF)

        zrow = pool.tile([128, F], mybir.dt.float32)
        nc.vector.memset(zrow[:, :], 0.0)

        # --- hybrid fixup placement. Samples 0-5: copy then scatter zeros
        # into `out` (scatter b gated on copy b; the last of these, b=5,
        # completes ~25us before the stream ends, fully hidden). Samples
        # 6-7: their copies do not READ x until the very end of the ring,
        # so instead pre-poison x[6], x[7] with zeros early (~20us) and
        # let the copies carry the zeros -- this removes the ~5us scatter
        # tail after the last copies. The tracker treats every dynamic AP
        # as touching the whole tensor; prune those conservative deps to
        # the true ones (cp6/cp7 after their x-scatter, out-scatter b
        # after cp_b).
        PRE = (6, 7)
        out_flat = out_d[:].rearrange("b t f -> (b t) f")
        x_flat = x_d[:].rearrange("b t f -> (b t) f")
        xscs = {}
        for b in PRE:
            sc = nc.gpsimd.indirect_dma_start(
                out=x_flat,
                out_offset=bass.IndirectOffsetOnAxis(ap=idx[:, b:b + 1], axis=0),
                in_=zrow[:, :],
                in_offset=None,
                bounds_check=BL * T - 1,
                oob_is_err=False,
            )
            xscs[b] = _raw(sc)
        cps = []
        for b in range(BL):
            eng = nc.sync if b % 2 == 0 else nc.scalar
            cp = eng.dma_start(
                out_d[b].flatten().unsqueeze(0),
                x_d[b].flatten().unsqueeze(0),
            )
            cps.append(_raw(cp))
        scs = {}
        for b in range(BL):
            if b in PRE:
                continue
            sc = nc.gpsimd.indirect_dma_start(
                out=out_flat,
                out_offset=bass.IndirectOffsetOnAxis(ap=idx[:, b:b + 1], axis=0),
                in_=zrow[:, :],
                in_offset=None,
                bounds_check=BL * T - 1,
                oob_is_err=False,
            )
            scs[b] = _raw(sc)
        cp_names = {c.name for c in cps}
        all_sc = {s.name for s in scs.values()} | {s.name for s in xscs.values()}
        for b, s in scs.items():
            keep = {cps[b].name}
            for n in list(s.sync_dependency_names()):
                if (n in cp_names or n in all_sc) and n not in keep:
                    s.try_remove_dependency(n)
        for b, s in xscs.items():
            for n in list(s.sync_dependency_names()):
                if n in cp_names or n in all_sc:
                    s.try_remove_dependency(n)
        for b, c in enumerate(cps):
            keep = {xscs[b].name} if b in PRE else set()
            for n in list(c.sync_dependency_names()):
                if n in all_sc and n not in keep:
                    c.try_remove_dependency(n)

    nc.compile()
    return nc


def _in_maps(x, bgn, distance):
    xs = np.ascontiguousarray(x, dtype=np.float32).reshape(B, T, F)
    bgn = np.ascontiguousarray(bgn, dtype=np.int32)
    distance = np.ascontiguousarray(distance, dtype=np.int32)
    tab = np.empty((128, 1 + BL), dtype=np.int32)
    tab[:, 0] = np.arange(128, dtype=np.int32)
    tab[:, 1:] = (np.arange(BL, dtype=np.int32) * T)[None, :]
    maps = []
    for i in range(N_CORES):
        sl = slice(i * BL, (i + 1) * BL)
        maps.append({
            "x": np.ascontiguousarray(xs[sl]),
            "bgn": np.ascontiguousarray(bgn[sl]),
            "distance": np.ascontiguousarray(distance[sl]),
            "tab": tab,
        })
    return maps


def _get_nc():
    global _cached_nc
    if _cached_nc is None:
        _cached_nc = _build()
    return _cached_nc


def kernel(x, bgn, distance):
    from concourse.bass_utils import run_bass_kernel_spmd

    nc = _get_nc()
    res = run_bass_kernel_spmd(nc, _in_maps(x, bgn, distance),
                               core_ids=list(range(N_CORES)))
    out = np.stack([res.results[i]["out"] for i in range(N_CORES)], axis=0)
    return out.reshape(B, C, T, F)
